# revision 3
# baseline (speedup 1.0000x reference)
"""Multi-head attention (B=4, S=2048, D=1024, H=16, HD=64) on 8 TRN2 NeuronCores.

Sharding: core c handles batch b=c//2 and head-group g=c%2 (8 heads).
W_q/W_k/W_v column-sharded, W_o row-sharded; the two partial outputs per
batch are summed on the host.

v2 restructure vs baseline:
  - One DMA per (tensor, i-block) / per weight tensor via a
    "(d p) s -> p d s" DRAM rearrange: 16 big input DMAs instead of ~100
    small ones. Avoids the ~625ns/DMA HWDGE serialization and reaches
    full DMA bandwidth; the first attention needs only ~6 MB.
  - x inputs stream per-i-block ([128, 4096] = all 8 d-tiles), double
    buffered: 48 KB/partition instead of 96 for full residency.
  - ib-major main loop: for each 512-query i-block, project k/q for each
    pair then run its attention; out-projection for i-block N is emitted
    during i-block N+1 (tail shrinks from ~27us to ~7us).
  - Score/ctx matmuls are emitted pairwise ([s(e) s(e+1) c(e) c(e+1)]) to
    halve PE tiling-mode switches (64-row-tiled scores vs full-array ctx).
  - Out-proj PSUM evacuation on DVE (ScalarE is the exp bottleneck).
  - reps>1 re-emits the whole body (same tiles) for HW slope timing.
"""

import sys

sys.path.insert(0, "/opt/trn_rl_repo")

import numpy as np
import ml_dtypes

import concourse.bacc as bacc
import concourse.tile as tile
from concourse import mybir

BF16 = ml_dtypes.bfloat16
F32 = mybir.dt.float32
BF = mybir.dt.bfloat16

B, S, D, H, HD = 4, 2048, 1024, 16, 64
G = 2              # head groups (cores per batch)
HPG = H // G       # 8 heads per group
NPAIR = HPG // 2   # 4 head pairs
FB = HPG * HD      # 512 projection cols per group
BLK = 128          # j-tile size
IBW = 512          # i-block width
NIB = S // IBW     # 4 i-blocks
NJT = S // BLK     # 16 j-tiles
NDT = D // BLK     # 8 contraction tiles
NST = S // BLK     # 16 s-tiles for the output projection
VW = HD + 1        # 65: v plus ones column
EXP_SCALE = 1.0 / np.sqrt(np.float32(HD))


def classify_mask(mask: np.ndarray):
    """Block states over the *transposed* mask grid: state[jt][it]:
    0=all valid, 1=all masked, 2=mixed."""
    m = np.asarray(mask)
    blocks = m.reshape(NJT, BLK, NJT, BLK).transpose(0, 2, 1, 3)  # [it, jt, i, j]
    anym = blocks.any(axis=(2, 3))
    allm = blocks.all(axis=(2, 3))
    states = np.where(allm, 1, np.where(anym, 2, 0)).astype(np.int8)
    return states.T  # index [jt, it]


def build_plan(states: np.ndarray):
    """Per i-block: list of (jt, c0, c1, mixed_ks)."""
    plan = []
    mixed_slots = {}
    for ib in range(NIB):
        its = list(range(4 * ib, 4 * ib + 4))
        jts = []
        for jt in range(NJT):
            sub = [int(states[jt, it]) for it in its]
            nz = [k for k, st in enumerate(sub) if st != 1]
            if not nz:
                continue
            k0, k1 = nz[0], nz[-1]
            mixed = [k for k in range(k0, k1 + 1) if sub[k] != 0]
            for k in mixed:
                mixed_slots.setdefault((jt, its[k]), len(mixed_slots))
            jts.append((jt, k0 * BLK, (k1 + 1) * BLK, mixed))
        assert jts, "fully-masked i-block not supported"
        plan.append(jts)
    return plan, mixed_slots


def plan_key(plan, mixed_slots):
    return (
        tuple(
            tuple((jt, c0, c1, tuple(mk)) for jt, c0, c1, mk in jts) for jts in plan
        ),
        tuple(sorted(mixed_slots.items())),
    )


def build_nc(plan, mixed_slots, reps=1):
    nvb = max(1, len(mixed_slots))
    nc = bacc.Bacc("TRN2", target_bir_lowering=False, debug=False, num_devices=8)

    xqT = nc.dram_tensor("xqT", [D, S], BF, kind="ExternalInput").ap()
    xkT = nc.dram_tensor("xkT", [D, S], BF, kind="ExternalInput").ap()
    xvT = nc.dram_tensor("xvT", [D, S], BF, kind="ExternalInput").ap()
    wq = nc.dram_tensor("wq", [D, FB], BF, kind="ExternalInput").ap()
    wk = nc.dram_tensor("wk", [D, FB], BF, kind="ExternalInput").ap()
    wv = nc.dram_tensor("wv", [D, FB], BF, kind="ExternalInput").ap()
    wo = nc.dram_tensor("wo", [FB, D], BF, kind="ExternalInput").ap()
    validT = nc.dram_tensor("validT", [nvb, BLK, BLK], BF, kind="ExternalInput").ap()
    out = nc.dram_tensor("out", [S, D], F32, kind="ExternalOutput").ap()

    with tile.TileContext(nc) as tc:
        import contextlib

        ctxmgr = contextlib.ExitStack()
        with ctxmgr:
            persist = ctxmgr.enter_context(tc.tile_pool(name="persist", bufs=1))
            xpool = ctxmgr.enter_context(tc.tile_pool(name="xpool", bufs=2))
            scp = ctxmgr.enter_context(tc.tile_pool(name="scp", bufs=2, space="PSUM"))
            projp = ctxmgr.enter_context(tc.tile_pool(name="projp", bufs=2, space="PSUM"))
            ctxp = ctxmgr.enter_context(tc.tile_pool(name="ctxp", bufs=2, space="PSUM"))
            atp = ctxmgr.enter_context(tc.tile_pool(name="atp", bufs=3))
            small = ctxmgr.enter_context(tc.tile_pool(name="small", bufs=4))
            drp = ctxmgr.enter_context(tc.tile_pool(name="drp", bufs=4, space="DRAM"))

            # ---- persistent tiles (allocated once; reps re-fill them) -----
            # weights packed d-major into one tile each: [128, (d f)]
            wv_t = persist.tile([BLK, NDT * FB], BF, name="wv_t")
            wk_t = persist.tile([BLK, NDT * FB], BF, name="wk_t")
            wq_t = persist.tile([BLK, NDT * FB], BF, name="wq_t")
            wo_t = persist.tile([BLK, NPAIR * D], BF, name="wo_t")
            valid_sb = persist.tile([BLK, nvb * BLK], BF, name="valid_sb")
            qT_sb = [persist.tile([BLK, S], BF, name=f"qT{p}") for p in range(NPAIR)]
            kT_sb = [persist.tile([BLK, S], BF, name=f"kT{p}") for p in range(NPAIR)]
            v_sb = [persist.tile([BLK, HPG * VW], BF, name=f"v{j}") for j in range(NJT)]
            ctxT_sb = [persist.tile([BLK, S], BF, name=f"cT{p}") for p in range(NPAIR)]

            for rep in range(reps):
                emit_body(
                    nc, plan, mixed_slots, nvb,
                    xqT, xkT, xvT, wq, wk, wv, wo, validT, out,
                    wv_t, wk_t, wq_t, wo_t, valid_sb,
                    qT_sb, kT_sb, v_sb, ctxT_sb,
                    xpool, scp, projp, ctxp, atp, small, drp, rep,
                )

    nc.compile()
    return nc


def emit_body(
    nc, plan, mixed_slots, nvb,
    xqT, xkT, xvT, wq, wk, wv, wo, validT, out,
    wv_t, wk_t, wq_t, wo_t, valid_sb,
    qT_sb, kT_sb, v_sb, ctxT_sb,
    xpool, scp, projp, ctxp, atp, small, drp, rep,
):
    r = f"r{rep}"

    # DRAM views: x as [p, d, s], weights as [p, d, f] / [p, pair, e]
    xv_v = xvT.rearrange("(d p) s -> p d s", p=BLK)
    xk_v = xkT.rearrange("(d p) s -> p d s", p=BLK)
    xq_v = xqT.rearrange("(d p) s -> p d s", p=BLK)

    # ---- input DMAs: one per (tensor, ib) + one per weight ------------
    # Order: v-side first (vproj gates everything), then k/q for ib0 so
    # the first attention can start after ~6 MB, then the rest.
    qs = [nc.sync, nc.scalar]

    def big(i, dst3, src3):
        qs[i % 2].dma_start(out=dst3, in_=src3)

    xv_c, xk_c, xq_c = [], [], []

    def load_x(ib, i0):
        cs = slice(ib * IBW, (ib + 1) * IBW)
        for i, (lst, nm, v) in enumerate(
            ((xv_c, "xv", xv_v), (xk_c, "xk", xk_v), (xq_c, "xq", xq_v))
        ):
            t = xpool.tile([BLK, NDT * IBW], BF, tag=nm, name=f"{nm}_{ib}_{r}")
            lst.append(t.rearrange("p (d s) -> p d s", s=IBW))
            big(i0 + i, lst[ib], v[:, :, cs])

    big(0, wv_t.rearrange("p (d f) -> p d f", f=FB), wv.rearrange("(d p) f -> p d f", p=BLK))
    load_x(0, 1)
    big(0, wk_t.rearrange("p (d f) -> p d f", f=FB), wk.rearrange("(d p) f -> p d f", p=BLK))
    big(1, wq_t.rearrange("p (d f) -> p d f", f=FB), wq.rearrange("(d p) f -> p d f", p=BLK))
    if nvb:
        nc.scalar.dma_start(
            out=valid_sb.rearrange("p (n f) -> p n f", f=BLK),
            in_=validT.rearrange("n p f -> p n f"),
        )
    for ib in range(1, NIB):
        load_x(ib, ib)
    nc.scalar.dma_start(
        out=wo_t.rearrange("p (q e) -> p q e", e=D),
        in_=wo.rearrange("(q p) e -> p q e", p=BLK),
    )

    # ---- compute helpers ---------------------------------------------
    def vproj_unit(j):
        sb = j // 4
        jc = slice((j % 4) * BLK, (j % 4 + 1) * BLK)
        ps = projp.tile([BLK, IBW], F32, tag="pp", name=f"vps{j}_{r}")
        for d in range(NDT):
            nc.tensor.matmul(
                ps,
                xv_c[sb][:, d, jc],
                wv_t[:, d * FB:(d + 1) * FB],
                start=(d == 0),
                stop=(d == NDT - 1),
            )
        dst = v_sb[j].rearrange("p (h w) -> p h w", w=VW)
        nc.vector.tensor_copy(dst[:, :, 0:HD], ps.rearrange("p (h w) -> p h w", w=HD))
        nc.vector.memset(dst[:, :, HD:VW], 1.0)

    def project(p, ib, w_t, x_c, dst_sb, nm):
        ps = projp.tile([BLK, IBW], F32, tag="pp", name=nm)
        for d in range(NDT):
            nc.tensor.matmul(
                ps,
                w_t[:, d * FB + p * BLK:d * FB + (p + 1) * BLK],
                x_c[ib][:, d, :],
                start=(d == 0),
                stop=(d == NDT - 1),
            )
        nc.vector.tensor_copy(dst_sb[p][:, ib * IBW:(ib + 1) * IBW], ps)

    filler = []

    def drain(n):
        for _ in range(min(n, len(filler))):
            filler.pop(0)[1]()

    def drain_kind(kind):
        keep = []
        for k, fn in filler:
            if k == kind:
                fn()
            else:
                keep.append((k, fn))
        filler[:] = keep

    def attention(p, ib):
        jts = plan[ib]
        ctx0 = ctxp.tile([VW, IBW], F32, tag="ctx", name=f"c0_{p}_{ib}_{r}")
        ctx1 = ctxp.tile([VW, IBW], F32, tag="ctx", name=f"c1_{p}_{ib}_{r}")
        nj = len(jts)
        sc_t = {}

        def emit_scores(e):
            jt, c0, c1, mixed = jts[e]
            w = c1 - c0
            sc = scp.tile([BLK, 2 * IBW], F32, tag="sc", name=f"s{p}_{ib}_{jt}_{r}")
            nc.tensor.matmul(
                sc[:, c0:c1],
                kT_sb[p][0:HD, jt * BLK:(jt + 1) * BLK],
                qT_sb[p][0:HD, ib * IBW + c0:ib * IBW + c1],
                start=True,
                stop=True,
            )
            nc.tensor.matmul(
                sc[:, IBW:IBW + w],
                kT_sb[p][HD:BLK, jt * BLK:(jt + 1) * BLK],
                qT_sb[p][HD:BLK, ib * IBW + c0:ib * IBW + c1],
                start=True,
                stop=True,
                tile_position=(HD, 0),
            )
            sc_t[e] = sc

        at_t = {}

        def emit_exp(e):
            jt, c0, c1, mixed = jts[e]
            w = c1 - c0
            sc = sc_t.pop(e)
            at = atp.tile([BLK, 2 * IBW], BF, tag="at", name=f"a{p}_{ib}_{jt}_{r}")
            nc.scalar.activation(
                out=at[:, c0:IBW + w],
                in_=sc[:, c0:IBW + w],
                func=mybir.ActivationFunctionType.Exp,
                scale=float(EXP_SCALE),
            )
            for k in mixed:
                slot = mixed_slots[(jt, 4 * ib + k)]
                vs = valid_sb[:, slot * BLK:(slot + 1) * BLK]
                nc.vector.tensor_mul(
                    at[:, k * BLK:(k + 1) * BLK],
                    at[:, k * BLK:(k + 1) * BLK],
                    vs,
                )
                h1c = IBW + k * BLK - c0
                nc.vector.tensor_mul(
                    at[:, h1c:h1c + BLK], at[:, h1c:h1c + BLK], vs
                )
            at_t[e] = at

        def emit_ctx(e):
            jt, c0, c1, mixed = jts[e]
            w = c1 - c0
            at = at_t.pop(e)
            vv = v_sb[jt].rearrange("p (h w) -> p h w", w=VW)
            nc.tensor.matmul(
                ctx0[:, c0:c1],
                vv[:, 2 * p, :],
                at[:, c0:c1],
                start=(e == 0),
                stop=(e == nj - 1),
            )
            nc.tensor.matmul(
                ctx1[:, c0:c1],
                vv[:, 2 * p + 1, :],
                at[:, IBW:IBW + w],
                start=(e == 0),
                stop=(e == nj - 1),
            )

        # Per 2-entry group: exps first (registers the sc-slot readers),
        # one filler unit (~4 independent matmuls) to absorb the exp wait,
        # then the next score pair (WAR on the sc slot), then both ctx
        # pairs. PE stream: s0 s1 | F | s2 s3 | c0 c1 | F | s4 s5 | ...
        # — score pairs stay adjacent (64-row-tiled mode), ctx+filler are
        # full-mode, so tiling-mode switches are halved vs per-entry.
        emit_scores(0)
        if nj > 1:
            emit_scores(1)
        for e0 in range(0, nj, 2):
            es = [e for e in (e0, e0 + 1) if e < nj]
            for e in es:
                emit_exp(e)
            drain(1)
            for e in es:
                if e + 2 < nj:
                    emit_scores(e + 2)
            for e in es:
                emit_ctx(e)

        # evacuation + normalization (DRAM-bounce), split per head into
        # two independent staggered chains (copy -> in-place recip ->
        # bounce -> broadcast -> half-mul) so the exposed latency at the
        # kernel end is one head's chain, not the merged row's.
        stg = small.tile([VW, 2 * IBW], BF, tag="stg", name=f"st{p}_{ib}_{r}")
        dd = drp.tile([2, IBW], BF, tag="dd", name=f"dd{p}_{ib}_{r}")
        rbc = small.tile([BLK, IBW], BF, tag="rbc", name=f"rb{p}_{ib}_{r}")
        blk = ctxT_sb[p][:, ib * IBW:(ib + 1) * IBW]
        for h, cps in ((0, ctx0), (1, ctx1)):
            hc = slice(h * IBW, (h + 1) * IBW)
            nc.vector.tensor_copy(stg[:, hc], cps)
            nc.sync.dma_start(
                out=ctxT_sb[p][h * HD:(h + 1) * HD, ib * IBW:(ib + 1) * IBW],
                in_=stg[0:HD, hc],
            )
            with nc.allow_low_precision(reason="bf16 1/denom, matches baseline"):
                nc.vector.reciprocal(out=stg[HD:VW, hc], in_=stg[HD:VW, hc])
            nc.sync.dma_start(out=dd[h:h + 1, :], in_=stg[HD:VW, hc])
            nc.sync.dma_start(
                out=rbc[h * HD:(h + 1) * HD, :],
                in_=dd[h:h + 1, :].partition_broadcast(HD),
            )
        for h in range(2):
            nc.vector.tensor_mul(
                blk[h * HD:(h + 1) * HD, :],
                blk[h * HD:(h + 1) * HD, :],
                rbc[h * HD:(h + 1) * HD, :],
            )

    def outproj_mms(po, st, nb, ps):
        for p in ps:
            nc.tensor.matmul(
                po,
                ctxT_sb[p][:, st * BLK:(st + 1) * BLK],
                wo_t[:, p * D + nb * IBW:p * D + (nb + 1) * IBW],
                start=(p == 0),
                stop=(p == NPAIR - 1),
            )

    def outproj_evac(po, st, nb):
        ot = small.tile([BLK, IBW], F32, tag="ot", name=f"ot{st}_{nb}_{r}")
        nc.vector.tensor_copy(ot, po)
        nc.scalar.dma_start(
            out=out[st * BLK:(st + 1) * BLK, nb * IBW:(nb + 1) * IBW],
            in_=ot,
        )

    def outproj_unit(st, nb):
        po = projp.tile([BLK, IBW], F32, tag="pp", name=f"po{st}_{nb}_{r}")
        outproj_mms(po, st, nb, range(NPAIR))
        outproj_evac(po, st, nb)

    def outproj_halves(st, nb):
        # two 2-matmul filler units sharing one accumulation group; A and
        # B are pushed adjacently so the 2-slot pp rotation never has to
        # wait on a B that sits later in the PE FIFO.
        holder = []

        def ua():
            po = projp.tile([BLK, IBW], F32, tag="pp", name=f"po{st}_{nb}_{r}")
            holder.append(po)
            outproj_mms(po, st, nb, (0, 1))

        def ub():
            po = holder[0]
            outproj_mms(po, st, nb, (2, 3))
            outproj_evac(po, st, nb)

        return ua, ub

    # ---- main schedule ------------------------------------------------
    # vproj(ib0) runs inline before the first attention; later vprojs and
    # all out-projections flow through the filler queue, drained one unit
    # per 2-entry attention group (PE bubble absorption). vproj units are
    # force-drained at each ib boundary (needed by the next ib's ctx).
    for j in range(4):
        vproj_unit(j)
    for ib in range(NIB):
        for p in range(NPAIR):
            if p == 0 and ib + 1 < NIB:
                for j in range(4 * (ib + 1), 4 * (ib + 1) + 4):
                    filler.append(("vproj", (lambda jj: lambda: vproj_unit(jj))(j)))
            if p == 1 and ib > 0:
                for st in range(4 * (ib - 1), 4 * (ib - 1) + 4):
                    for nb in range(2):
                        if ib == NIB - 1:
                            # ib3 has 32 drain slots but little supply:
                            # split its outproj fillers into halves
                            ua, ub = outproj_halves(st, nb)
                            filler.append(("outproj", ua))
                            filler.append(("outproj", ub))
                        else:
                            filler.append(
                                ("outproj", (lambda s_, n_: lambda: outproj_unit(s_, n_))(st, nb))
                            )
            if p == 0 and ib == 0:
                project(0, 0, wk_t, xk_c, kT_sb, f"kps0_0_{r}")
                project(0, 0, wq_t, xq_c, qT_sb, f"qps0_0_{r}")
            # queue next pair's k/q projections as filler (emitted inside
            # this attention; force-drained at its end)
            np_, nib = (p + 1, ib) if p + 1 < NPAIR else (0, ib + 1)
            if nib < NIB:
                for w_t, x_c, dst, nm in (
                    (wk_t, xk_c, kT_sb, f"kps{np_}_{nib}_{r}"),
                    (wq_t, xq_c, qT_sb, f"qps{np_}_{nib}_{r}"),
                ):
                    # deadline-critical: next pair's scores stall on these,
                    # so they jump the filler queue
                    filler.insert(
                        len([u for u in filler if u[0] == "kqps"]),
                        ("kqps", (lambda a, b, c, d, e2, f2: lambda: project(a, b, c, d, e2, f2))(
                            np_, nib, w_t, x_c, dst, nm))
                    )
            attention(p, ib)
            drain_kind("kqps")
        drain_kind("vproj")
    drain_kind("fin")
    drain_kind("outproj")
    for st in range(4 * (NIB - 1), 4 * (NIB - 1) + 4):
        for nb in range(2):
            outproj_unit(st, nb)


_BUILD_CACHE: dict = {}


def _get_nc(mask: np.ndarray, reps=1):
    states = classify_mask(mask)
    plan, mixed_slots = build_plan(states)
    key = (plan_key(plan, mixed_slots), reps)
    if key not in _BUILD_CACHE:
        _BUILD_CACHE[key] = (build_nc(plan, mixed_slots, reps), plan, mixed_slots)
    return _BUILD_CACHE[key]


def _make_in_maps(xq, xk, xv, mask, W_q, W_k, W_v, W_o, mixed_slots):
    nvb = max(1, len(mixed_slots))
    vt = np.zeros((nvb, BLK, BLK), BF16)
    m = np.asarray(mask)
    for (jt, it), slot in mixed_slots.items():
        vt[slot] = (~m[it * BLK:(it + 1) * BLK, jt * BLK:(jt + 1) * BLK]).T.astype(BF16)
    xT = {}
    for b in range(B):
        xT[b] = tuple(
            np.asarray(x[b]).T.astype(BF16) for x in (xq, xk, xv)
        )
    in_maps = []
    for c in range(8):
        b, g = c // G, c % G
        cols = slice(g * FB, (g + 1) * FB)
        in_maps.append(
            {
                "xqT": xT[b][0],
                "xkT": xT[b][1],
                "xvT": xT[b][2],
                "wq": np.asarray(W_q)[:, cols].astype(BF16),
                "wk": np.asarray(W_k)[:, cols].astype(BF16),
                "wv": np.asarray(W_v)[:, cols].astype(BF16),
                "wo": np.asarray(W_o)[cols, :].astype(BF16),
                "validT": vt,
            }
        )
    return in_maps


PROFILE = False
last_hw_exec_ns = None


def kernel(xq, xk, xv, mask, W_q, W_k, W_v, W_o):
    global last_hw_exec_ns
    from concourse import bass_utils

    nc, plan, mixed_slots = _get_nc(mask)
    in_maps = _make_in_maps(xq, xk, xv, mask, W_q, W_k, W_v, W_o, mixed_slots)
    res = bass_utils.run_bass_kernel_spmd(
        nc, in_maps, core_ids=list(range(8))
    )
    if res.exec_time_ns:
        last_hw_exec_ns = res.exec_time_ns
    out = np.empty((B, S, D), np.float32)
    for b in range(B):
        out[b] = res.results[2 * b]["out"] + res.results[2 * b + 1]["out"]
    return out

